# revision 9
# baseline (speedup 1.0000x reference)
"""Single-qubit Kraus channel on a batched density matrix, on 8 trn2 cores.

rho -> sum_k K_k rho K_k^dagger applied to one target qubit of an n-qubit
density matrix state[2^n, 2^n, B].

The two-sided contraction reduces to a 4x4 coefficient matrix
    C[p,q,i,j] = sum_k K[k,p,i] * conj(K[k,q,j])
acting block-wise: out(row-bit p, col-bit q) = sum_ij C[p,q,i,j] * in(i, j),
i.e. every output element is a <=4-term linear combination of input elements
that differ only in the target bit of the row/column index.  Pure memory
bound: read input once, write output once.

Sharding: data-parallel over contiguous row blocks (4096 rows -> 512/core).
Per core, tiles of [128 partitions x 4096 f32] pair the target-row-bit
halves on identical partitions so all compute is lane-aligned:
  partition p = a_local*64 + r  <->  dram row g*256 + a_local*128 + i*64 + r
Compute per output block: ScalarE scaled copy for the smallest term,
VectorE scalar_tensor_tensor (fused (x*c)+y) for the rest.
"""

import sys

_REPO = "/opt/trn_rl_repo"
if _REPO not in sys.path:
    sys.path.insert(0, _REPO)

import numpy as np

import concourse.bacc as bacc
import concourse.bass as bass
import concourse.mybir as mybir
from concourse.bass_utils import run_bass_kernel_spmd
from concourse.tile import TileContext

N_CORES = 8

# Graded configuration (reference.setup_inputs): n=12, target=5, B=4.
DIM = 4096
BATCH = 4
ROWS = DIM // N_CORES          # 512 rows per core
FREE = DIM * BATCH             # 16384 f32 per row
R_ROW = 64                     # rows right of target bit (row side)
RB = R_ROW * BATCH             # 256 f32: one col-side j-block
CGRP = 2 * RB                  # 512 f32: one col group (j=0 half + j=1 half)
W = 4096                       # chunk: f32 per partition per tile (8 col groups)
NW = FREE // W                 # 4 chunks
NG = ROWS // (4 * R_ROW)       # 2 supergroups of 256 rows (two 128-row a-groups)

_COEF_TOL = 0.0  # exact-zero test; bit-flip channel cross terms are exact 0s

_prog_cache: dict = {}


def _build_program(coefs: tuple, repeat: int = 1) -> "bass.Bass":
    """Build the per-core SPMD program for coefficient matrix C[p,q,i,j].

    repeat > 1 wraps the whole body in a hardware loop — benchmarking only
    (recomputes the same output repeat times).
    """
    f32 = mybir.dt.float32
    ncg = W // CGRP  # col groups per chunk

    nc = bacc.Bacc("TRN2", target_bir_lowering=False, debug=False)
    x = nc.dram_tensor("x", [ROWS, FREE], f32, kind="ExternalInput")
    y = nc.dram_tensor("y", [ROWS, FREE], f32, kind="ExternalOutput")
    # row = ((g*2 + a)*2 + i)*64 + r
    xr = x.rearrange("(g a i r) c -> g a i r c", g=NG, a=2, i=2, r=R_ROW)
    yr = y.rearrange("(g a p r) c -> g a p r c", g=NG, a=2, p=2, r=R_ROW)

    def jview(tile, j):
        # [128, ncg, RB] view selecting the col-side j half of every col group
        return tile.rearrange("p (c j t) -> p c j t", j=2, t=RB)[:, :, j, :]

    from contextlib import ExitStack

    with TileContext(nc) as tc, ExitStack() as stack:
        if repeat > 1:
            stack.enter_context(tc.For_i(0, repeat, 1))
        with tc.tile_pool(name="xin", bufs=2) as px, \
             tc.tile_pool(name="yout", bufs=2) as po:
            for g in range(NG):
                for w in range(NW):
                    cs = slice(w * W, (w + 1) * W)
                    xt = []
                    for i in (0, 1):
                        t = px.tile([128, W], f32, tag=f"x{i}")
                        # DRAM side is [a=2, r=64, W]; SBUF side [128, W].
                        # The DMA linearizes both sides in order, so the
                        # (a r) pair lands on partitions 0..127.
                        nc.sync.dma_start(out=t[:], in_=xr[g, :, i, :, cs])
                        xt.append(t)
                    ot = []
                    for p in (0, 1):
                        t = po.tile([128, W], f32, tag=f"o{p}")
                        ot.append(t)
                        for q in (0, 1):
                            ov = jview(t, q)
                            terms = [
                                (coefs[((p * 2 + q) * 2 + i) * 2 + j], i, j)
                                for i in (0, 1)
                                for j in (0, 1)
                                if abs(coefs[((p * 2 + q) * 2 + i) * 2 + j])
                                > _COEF_TOL
                            ]
                            terms.sort(key=lambda it: -abs(it[0]))
                            if not terms:
                                nc.vector.memset(ov, 0.0)
                                continue
                            # smallest |coef| term lands in ov via ScalarE,
                            # remaining terms accumulate in place via
                            # VectorE fused (x*c)+y.  In-place keeps each
                            # instruction's semaphore-wait count low (the
                            # STT encoding has few sync-wait slots).
                            cn, i_n, j_n = terms[-1]
                            nc.scalar.mul(ov, jview(xt[i_n], j_n), cn)
                            for ck, ik, jk in terms[-2::-1]:
                                nc.vector.scalar_tensor_tensor(
                                    out=ov,
                                    in0=jview(xt[ik], jk),
                                    scalar=float(ck),
                                    in1=ov,
                                    op0=mybir.AluOpType.mult,
                                    op1=mybir.AluOpType.add,
                                )
                    for p in (0, 1):
                        nc.sync.dma_start(out=yr[g, :, p, :, cs], in_=ot[p][:])
    nc.compile()
    return nc


def _fallback(state, C, L, R, B):
    rho = state.reshape(L, 2, R, L, 2, R, B)
    out = np.einsum("pqij,aibcjdz->apbcqdz", C, rho.astype(np.float64))
    return out.reshape(state.shape).astype(state.dtype)


def kernel(state, kraus, target, n_qubits, _profile=False):
    state = np.asarray(state)
    kraus = np.asarray(kraus)
    t = int(np.asarray(target))
    n = int(np.asarray(n_qubits))
    dim = 1 << n
    B = state.shape[-1]
    L = 1 << t
    R = dim // (2 * L)

    C = np.einsum(
        "kpi,kqj->pqij",
        kraus.astype(np.float64),
        np.conj(kraus).astype(np.float64),
    )

    if not (
        state.shape == (DIM, DIM, BATCH)
        and state.dtype == np.float32
        and R == R_ROW
        and L * 2 * R == DIM
    ):
        return _fallback(state, C, L, R, B)

    coefs = tuple(float(v) for v in C.reshape(-1))
    nc = _prog_cache.get(coefs)
    if nc is None:
        nc = _build_program(coefs)
        _prog_cache[coefs] = nc

    flat = state.reshape(DIM, FREE)
    in_maps = [
        {"x": flat[c * ROWS : (c + 1) * ROWS]} for c in range(N_CORES)
    ]
    res = run_bass_kernel_spmd(
        nc, in_maps, core_ids=list(range(N_CORES)), trace=_profile
    )
    out = np.concatenate([res.results[c]["y"] for c in range(N_CORES)], axis=0)
    out = out.reshape(DIM, DIM, BATCH)
    if _profile:
        return out, res
    return out


# revision 10
# speedup vs baseline: 8.2490x; 8.2490x over previous
"""Single-qubit Kraus channel on a batched density matrix, on 8 trn2 cores.

rho -> sum_k K_k rho K_k^dagger applied to one target qubit of an n-qubit
density matrix state[2^n, 2^n, B].

The two-sided contraction reduces to a 4x4 coefficient matrix
    C[p,q,i,j] = sum_k K[k,p,i] * conj(K[k,q,j])
acting block-wise: out(row-bit p, col-bit q) = sum_ij C[p,q,i,j] * in(i, j),
i.e. every output element is a <=4-term linear combination of input elements
that differ only in the target bit of the row/column index.  Pure memory
bound: read input once, write output once.

Sharding: data-parallel over contiguous row blocks (4096 rows -> 512/core).
Per core, tiles of [128 partitions x 4096 f32] pair the target-row-bit
halves on identical partitions so all compute is lane-aligned:
  partition p = a_local*64 + r  <->  dram row g*256 + a_local*128 + i*64 + r
Compute per output block: ScalarE scaled copy for the smallest term,
VectorE scalar_tensor_tensor (fused (x*c)+y) for the rest.
"""

import sys

_REPO = "/opt/trn_rl_repo"
if _REPO not in sys.path:
    sys.path.insert(0, _REPO)

import numpy as np

import concourse.bacc as bacc
import concourse.bass as bass
import concourse.mybir as mybir
from concourse.bass_utils import run_bass_kernel_spmd
from concourse.tile import TileContext

N_CORES = 8

# Graded configuration (reference.setup_inputs): n=12, target=5, B=4.
DIM = 4096
BATCH = 4
ROWS = DIM // N_CORES          # 512 rows per core
FREE = DIM * BATCH             # 16384 f32 per row
R_ROW = 64                     # rows right of target bit (row side)
RB = R_ROW * BATCH             # 256 f32: one col-side j-block
CGRP = 2 * RB                  # 512 f32: one col group (j=0 half + j=1 half)
W = 4096                       # chunk: f32 per partition per tile (8 col groups)
NW = FREE // W                 # 4 chunks
NG = ROWS // (4 * R_ROW)       # 2 supergroups of 256 rows (two 128-row a-groups)

_COEF_TOL = 0.0  # exact-zero test; bit-flip channel cross terms are exact 0s

_prog_cache: dict = {}


def _build_program(coefs: tuple, repeat: int = 1) -> "bass.Bass":
    """Build the per-core SPMD program for coefficient matrix C[p,q,i,j].

    repeat > 1 wraps the whole body in a hardware loop — benchmarking only
    (recomputes the same output repeat times).
    """
    f32 = mybir.dt.float32
    n_agrp = ROWS // 128  # natural 128-row groups per core

    nc = bacc.Bacc("TRN2", target_bir_lowering=False, debug=False)
    x = nc.dram_tensor("x", [ROWS, FREE], f32, kind="ExternalInput")
    y = nc.dram_tensor("y", [ROWS, FREE], f32, kind="ExternalOutput")

    def pjview(tile, p, j):
        # [64, ncg, RB]: partition half p (row target-bit), col-side j half
        # of every col group.
        return tile[p * 64 : (p + 1) * 64].rearrange(
            "p (c j t) -> p c j t", j=2, t=RB
        )[:, :, j, :]

    from contextlib import ExitStack

    with TileContext(nc) as tc, ExitStack() as stack:
        if repeat > 1:
            stack.enter_context(tc.For_i(0, repeat, 1))
        with tc.tile_pool(name="xin", bufs=2) as px, \
             tc.tile_pool(name="yout", bufs=2) as po:
            for a in range(n_agrp):
                rs = slice(a * 128, (a + 1) * 128)
                for w in range(NW):
                    cs = slice(w * W, (w + 1) * W)
                    xt = px.tile([128, W], f32, tag="x")
                    # 128 consecutive DRAM rows -> 128 partitions; fully
                    # contiguous 16 KiB runs per partition (fast DMA path).
                    # Partitions 0-63 hold target-row-bit 0, 64-127 bit 1.
                    nc.sync.dma_start(out=xt[:], in_=x[rs, cs])
                    ot = po.tile([128, W], f32, tag="o")
                    for p in (0, 1):
                        for q in (0, 1):
                            ov = pjview(ot, p, q)
                            terms = [
                                (coefs[((p * 2 + q) * 2 + i) * 2 + j], i, j)
                                for i in (0, 1)
                                for j in (0, 1)
                                if abs(coefs[((p * 2 + q) * 2 + i) * 2 + j])
                                > _COEF_TOL
                            ]
                            terms.sort(key=lambda it: -abs(it[0]))
                            if not terms:
                                nc.vector.memset(ov, 0.0)
                                continue
                            # smallest |coef| term lands in ov via ScalarE,
                            # remaining terms accumulate in place via
                            # VectorE fused (x*c)+y.  In-place keeps each
                            # instruction's semaphore-wait count low (the
                            # STT encoding has few sync-wait slots).
                            # Reads with i != p are cross-partition-offset
                            # (verified supported on HW).
                            cn, i_n, j_n = terms[-1]
                            nc.scalar.mul(ov, pjview(xt, i_n, j_n), cn)
                            for ck, ik, jk in terms[-2::-1]:
                                nc.vector.scalar_tensor_tensor(
                                    out=ov,
                                    in0=pjview(xt, ik, jk),
                                    scalar=float(ck),
                                    in1=ov,
                                    op0=mybir.AluOpType.mult,
                                    op1=mybir.AluOpType.add,
                                )
                    nc.sync.dma_start(out=y[rs, cs], in_=ot[:])
    nc.compile()
    return nc


def _fallback(state, C, L, R, B):
    rho = state.reshape(L, 2, R, L, 2, R, B)
    out = np.einsum("pqij,aibcjdz->apbcqdz", C, rho.astype(np.float64))
    return out.reshape(state.shape).astype(state.dtype)


def kernel(state, kraus, target, n_qubits, _profile=False):
    state = np.asarray(state)
    kraus = np.asarray(kraus)
    t = int(np.asarray(target))
    n = int(np.asarray(n_qubits))
    dim = 1 << n
    B = state.shape[-1]
    L = 1 << t
    R = dim // (2 * L)

    C = np.einsum(
        "kpi,kqj->pqij",
        kraus.astype(np.float64),
        np.conj(kraus).astype(np.float64),
    )

    if not (
        state.shape == (DIM, DIM, BATCH)
        and state.dtype == np.float32
        and R == R_ROW
        and L * 2 * R == DIM
    ):
        return _fallback(state, C, L, R, B)

    coefs = tuple(float(v) for v in C.reshape(-1))
    nc = _prog_cache.get(coefs)
    if nc is None:
        nc = _build_program(coefs)
        _prog_cache[coefs] = nc

    flat = state.reshape(DIM, FREE)
    in_maps = [
        {"x": flat[c * ROWS : (c + 1) * ROWS]} for c in range(N_CORES)
    ]
    res = run_bass_kernel_spmd(
        nc, in_maps, core_ids=list(range(N_CORES)), trace=_profile
    )
    out = np.concatenate([res.results[c]["y"] for c in range(N_CORES)], axis=0)
    out = out.reshape(DIM, DIM, BATCH)
    if _profile:
        return out, res
    return out


# revision 13
# speedup vs baseline: 9.2949x; 1.1268x over previous
"""Single-qubit Kraus channel on a batched density matrix, on 8 trn2 cores.

rho -> sum_k K_k rho K_k^dagger applied to one target qubit of an n-qubit
density matrix state[2^n, 2^n, B].

The two-sided contraction reduces to a 4x4 coefficient matrix
    C[p,q,i,j] = sum_k K[k,p,i] * conj(K[k,q,j])
acting block-wise: out(row-bit p, col-bit q) = sum_ij C[p,q,i,j] * in(i, j),
i.e. every output element is a <=4-term linear combination of input elements
that differ only in the target bit of the row/column index.  Pure memory
bound: read input once, write output once.

Sharding: data-parallel over contiguous row blocks (4096 rows -> 512/core).
Per core, tiles of [128 partitions x 4096 f32] pair the target-row-bit
halves on identical partitions so all compute is lane-aligned:
  partition p = a_local*64 + r  <->  dram row g*256 + a_local*128 + i*64 + r
Compute per output block: ScalarE scaled copy for the smallest term,
VectorE scalar_tensor_tensor (fused (x*c)+y) for the rest.
"""

import sys

_REPO = "/opt/trn_rl_repo"
if _REPO not in sys.path:
    sys.path.insert(0, _REPO)

import numpy as np

import concourse.bacc as bacc
import concourse.bass as bass
import concourse.mybir as mybir
from concourse.bass_utils import run_bass_kernel_spmd
from concourse.tile import TileContext

N_CORES = 8

# Graded configuration (reference.setup_inputs): n=12, target=5, B=4.
DIM = 4096
BATCH = 4
ROWS = DIM // N_CORES          # 512 rows per core
FREE = DIM * BATCH             # 16384 f32 per row
R_ROW = 64                     # rows right of target bit (row side)
RB = R_ROW * BATCH             # 256 f32: one col-side j-block
CGRP = 2 * RB                  # 512 f32: one col group (j=0 half + j=1 half)
W = 4096                       # chunk: f32 per partition per tile (8 col groups)
NW = FREE // W                 # 4 chunks
NG = ROWS // (4 * R_ROW)       # 2 supergroups of 256 rows (two 128-row a-groups)

_COEF_TOL = 0.0  # exact-zero test; bit-flip channel cross terms are exact 0s

_prog_cache: dict = {}


def _build_program(
    coefs: tuple,
    repeat: int = 1,
    tile_w: int = W,
    store_engine: str = "sync",
    bufs: int = 2,
) -> "bass.Bass":
    """Build the per-core SPMD program for coefficient matrix C[p,q,i,j].

    repeat > 1 wraps the whole body in a hardware loop — benchmarking only
    (recomputes the same output repeat times).
    """
    f32 = mybir.dt.float32
    W_ = tile_w
    NW_ = FREE // W_
    n_agrp = ROWS // 128  # natural 128-row groups per core

    nc = bacc.Bacc("TRN2", target_bir_lowering=False, debug=False)
    x = nc.dram_tensor("x", [ROWS, FREE], f32, kind="ExternalInput")
    y = nc.dram_tensor("y", [ROWS, FREE], f32, kind="ExternalOutput")

    def pjview(tile, p, j):
        # [64, ncg, RB]: partition half p (row target-bit), col-side j half
        # of every col group.
        return tile[p * 64 : (p + 1) * 64].rearrange(
            "p (c j t) -> p c j t", j=2, t=RB
        )[:, :, j, :]

    from contextlib import ExitStack

    with TileContext(nc) as tc, ExitStack() as stack:
        if repeat > 1:
            stack.enter_context(tc.For_i(0, repeat, 1))
        with tc.tile_pool(name="xin", bufs=bufs) as px, \
             tc.tile_pool(name="yout", bufs=bufs) as po:
            for a in range(n_agrp):
                rs = slice(a * 128, (a + 1) * 128)
                for w in range(NW_):
                    cs = slice(w * W_, (w + 1) * W_)
                    xt = px.tile([128, W_], f32, tag="x")
                    # 128 consecutive DRAM rows -> 128 partitions; fully
                    # contiguous 16 KiB runs per partition (fast DMA path).
                    # Partitions 0-63 hold target-row-bit 0, 64-127 bit 1.
                    nc.sync.dma_start(out=xt[:], in_=x[rs, cs])
                    ot = po.tile([128, W_], f32, tag="o")
                    for p in (0, 1):
                        for q in (0, 1):
                            ov = pjview(ot, p, q)
                            terms = [
                                (coefs[((p * 2 + q) * 2 + i) * 2 + j], i, j)
                                for i in (0, 1)
                                for j in (0, 1)
                                if abs(coefs[((p * 2 + q) * 2 + i) * 2 + j])
                                > _COEF_TOL
                            ]
                            terms.sort(key=lambda it: -abs(it[0]))
                            if not terms:
                                nc.vector.memset(ov, 0.0)
                                continue
                            # smallest |coef| term lands in ov via ScalarE,
                            # remaining terms accumulate in place via
                            # VectorE fused (x*c)+y.  In-place keeps each
                            # instruction's semaphore-wait count low (the
                            # STT encoding has few sync-wait slots).
                            # Reads with i != p are cross-partition-offset
                            # (verified supported on HW).
                            cn, i_n, j_n = terms[-1]
                            nc.scalar.mul(ov, pjview(xt, i_n, j_n), cn)
                            for ck, ik, jk in terms[-2::-1]:
                                nc.vector.scalar_tensor_tensor(
                                    out=ov,
                                    in0=pjview(xt, ik, jk),
                                    scalar=float(ck),
                                    in1=ov,
                                    op0=mybir.AluOpType.mult,
                                    op1=mybir.AluOpType.add,
                                )
                    getattr(nc, store_engine).dma_start(
                        out=y[rs, cs], in_=ot[:]
                    )
    nc.compile()
    return nc


def _fallback(state, C, L, R, B):
    rho = state.reshape(L, 2, R, L, 2, R, B)
    out = np.einsum("pqij,aibcjdz->apbcqdz", C, rho.astype(np.float64))
    return out.reshape(state.shape).astype(state.dtype)


def kernel(state, kraus, target, n_qubits, _profile=False):
    state = np.asarray(state)
    kraus = np.asarray(kraus)
    t = int(np.asarray(target))
    n = int(np.asarray(n_qubits))
    dim = 1 << n
    B = state.shape[-1]
    L = 1 << t
    R = dim // (2 * L)

    C = np.einsum(
        "kpi,kqj->pqij",
        kraus.astype(np.float64),
        np.conj(kraus).astype(np.float64),
    )

    if not (
        state.shape == (DIM, DIM, BATCH)
        and state.dtype == np.float32
        and R == R_ROW
        and L * 2 * R == DIM
    ):
        return _fallback(state, C, L, R, B)

    coefs = tuple(float(v) for v in C.reshape(-1))
    nc = _prog_cache.get(coefs)
    if nc is None:
        nc = _build_program(coefs)
        _prog_cache[coefs] = nc

    flat = state.reshape(DIM, FREE)
    in_maps = [
        {"x": flat[c * ROWS : (c + 1) * ROWS]} for c in range(N_CORES)
    ]
    res = run_bass_kernel_spmd(
        nc, in_maps, core_ids=list(range(N_CORES)), trace=_profile
    )
    out = np.concatenate([res.results[c]["y"] for c in range(N_CORES)], axis=0)
    out = out.reshape(DIM, DIM, BATCH)
    if _profile:
        return out, res
    return out
